# revision 1
# baseline (speedup 1.0000x reference)
"""NsNet2 single-step (fc1 + 2x GRU cell + 3x FC) Trainium2 kernel.

Strategy:
  - Pure data parallel: batch B=32768 sharded as 4096 rows per NeuronCore (8 cores).
  - Feature-major ("transposed") layout on chip: activations live as [feat, batch]
    so every matmul's moving operand is already in [K, N] form -> zero on-chip
    transposes. Host transposes inputs/outputs (free; not on HW critical path).
  - bf16 matmuls (full PE rate) with fp32 PSUM accumulation; fp32 biases fused
    into ScalarE activation (sigmoid/tanh) or VectorE tensor_scalar (relu).
  - fc1 is folded into the GRU1 input-gate weights on the host (fc1 is linear and
    f1 is consumed only by GRU1's input matmuls):  (x@Wfc1.T+b) @ Wg.T =
    x @ (Wg@Wfc1).T + (Wg@b + bg).
  - z,r gates sum their input-side and hidden-side matmuls in one PSUM, so their
    contraction operands are K-concatenated ([x|h1] resp. [g1|h2]) on the host /
    on chip, saving ceil() waste: GRU1 zr K=657->6 chunks (vs 3+4), GRU2
    K=800->7 (vs 4+4).
  - Feature dims zero-padded to multiples of 128 where needed; padding never
    increases PE chunk count and keeps matmul contraction at 128 partitions.
"""

import os
import sys

import numpy as np
import ml_dtypes

sys.path.insert(0, "/opt/trn_rl_repo")

import concourse.bacc as bacc
import concourse.bass as bass
import concourse.mybir as mybir
import concourse.tile as tile
from concourse.bass import ts
from concourse.bass_utils import run_bass_kernel_spmd

BF16 = ml_dtypes.bfloat16
FP8 = ml_dtypes.float8_e4m3

B, F, H, FF = 32768, 257, 400, 600
NCORES = 8
BPC = B // NCORES          # 4096 batch rows per core
Hp, FFp, Fp = 512, 640, 384  # padded feature dims
XH1 = 769                  # [x(257) | h1(400) | pad(112)] rows; 6 zr chunks + aligned h1 view at 257
ZR2K = 896                 # [g1(400) | h2(400) | pad(96)] -> 7 chunks
ZRM = 800                  # contiguous [z(400) | r(400)] output cols -> 7 M chunks
ZRC = 7
NB = 512                   # matmul free-dim tile (one PSUM bank of fp32)

AF = mybir.ActivationFunctionType
ALU = mybir.AluOpType

# packed bias column layout: name -> (offset, n_chunks)
BIAS_LAYOUT = {}
_off = 0
for _n, _c in (("bzr1", 7), ("bnx1", 4), ("bnh1", 4),
               ("bzr2", 7), ("bnx2", 4), ("bnh2", 4),
               ("bfc2", 5), ("bfc3", 5), ("bfc4", 3)):
    BIAS_LAYOUT[_n] = (_off, _c)
    _off += _c
BIAS_COLS = _off


def _pad2(a, rows, cols):
    out = np.zeros((rows, cols), dtype=np.float64)
    out[: a.shape[0], : a.shape[1]] = a
    return out


def _bias_tile(vec, padded):
    """Pack a [padded] bias vector as [128, padded//128] fp32 (partition-major)."""
    v = np.zeros(padded, dtype=np.float64)
    v[: vec.shape[0]] = vec
    return np.ascontiguousarray(v.reshape(padded // 128, 128).T).astype(np.float32)


def prepare_weights(inp):
    f64 = {k: np.asarray(v, dtype=np.float64) for k, v in inp.items()}
    w = {}

    # fc1 fold for GRU1 input side
    Wx = {}
    bx = {}
    for name in ("z", "r", "n"):
        Wx[name] = (f64[f"Wi{name}1"] @ f64["Wfc1"]).T          # [F, H]
        bx[name] = f64[f"bi{name}1"] + f64[f"Wi{name}1"] @ f64["bfc1"]

    # GRU1 z,r: K-concat [x(257) | h1(400)] -> rows 0..656 of XH1 space,
    # M = contiguous [z(400) | r(400)] = 800 -> 7 chunks; r is lane-realigned
    # on chip by a small SBUF->SBUF DMA after the sigmoid.
    Wzr1 = np.zeros((768, ZRM), dtype=np.float64)
    for g, name in enumerate(("z", "r")):
        Wzr1[:F, g * H : g * H + H] = Wx[name]
        Wzr1[F : F + H, g * H : g * H + H] = f64[f"Wh{name}1"].T
    w["Wzr1"] = Wzr1
    # GRU1 n input side: K = x chunks of XH1 (rows 0..383; rows 257+ are h1 -> zero)
    w["Wn1x"] = _pad2(Wx["n"], Fp, Hp)
    # GRU1 n hidden side: aligned h1 (XH1 rows 257..768)
    w["Wn1h"] = _pad2(f64["Whn1"].T, Hp, Hp)

    # GRU2 z,r: K-concat [g1(400) | h2(400)] -> 800 rows -> 7 chunks
    Wzr2 = np.zeros((ZR2K, ZRM), dtype=np.float64)
    for g, name in enumerate(("z", "r")):
        Wzr2[:H, g * H : g * H + H] = f64[f"Wi{name}2"].T
        Wzr2[H : 2 * H, g * H : g * H + H] = f64[f"Wh{name}2"].T
    w["Wzr2"] = Wzr2
    # GRU2 n input side: K = g1 aligned (4 chunks; chunk 3 partitions 16.. are h2 -> zero)
    w["Wn2x"] = _pad2(f64["Win2"].T, Hp, Hp)
    w["Wn2h"] = _pad2(f64["Whn2"].T, Hp, Hp)

    w["Wfc2T"] = _pad2(f64["Wfc2"].T, Hp, FFp)    # [512, 640]
    w["Wfc3T"] = _pad2(f64["Wfc3"].T, FFp, FFp)   # [640, 640]
    w["Wfc4T"] = _pad2(f64["Wfc4"].T, FFp, Fp)    # [640, 384]

    fp8_names = {"Wzr1", "Wn1x", "Wn1h", "Wzr2", "Wn2x", "Wn2h"}
    weights = {
        k: np.ascontiguousarray(v).astype(FP8 if k in fp8_names else BF16)
        for k, v in w.items()
    }

    parts = [
        ("bzr1", _bias_tile(np.concatenate([bx["z"] + f64["bhz1"],
                                            bx["r"] + f64["bhr1"]]), 896)),
        ("bnx1", _bias_tile(bx["n"], Hp)),
        ("bnh1", _bias_tile(f64["bhn1"], Hp)),
        ("bzr2", _bias_tile(np.concatenate([f64["biz2"] + f64["bhz2"],
                                            f64["bir2"] + f64["bhr2"]]), 896)),
        ("bnx2", _bias_tile(f64["bin2"], Hp)),
        ("bnh2", _bias_tile(f64["bhn2"], Hp)),
        ("bfc2", _bias_tile(f64["bfc2"], FFp)),
        ("bfc3", _bias_tile(f64["bfc3"], FFp)),
        ("bfc4", _bias_tile(f64["bfc4"], Fp)),
    ]
    biases = {"biasT": np.concatenate([p[1] for p in parts], axis=1)}
    return weights, biases


def build_nc(nbt=BPC, nb=NB):
    """Build the per-core Bass program. nbt = per-core batch, nb = free-dim tile."""
    nc = bacc.Bacc("TRN2", target_bir_lowering=False, debug=False)
    bf = mybir.dt.bfloat16
    f32 = mybir.dt.float32

    f8 = mybir.dt.float8e4

    # xh8 rows: 0..256 = x.T, 257..656 = h1.T, 657..768 = zeros (fp8 matmul
    # operand). zr view = rows 0..767 (6 chunks); aligned-h1 view = 257..768.
    xh8 = nc.declare_dram_parameter("xh8", [XH1, nbt], f8, isOutput=False)
    h1T = nc.declare_dram_parameter("h1T", [Hp, nbt], bf, isOutput=False)
    h2T = nc.declare_dram_parameter("h2T", [Hp, nbt], bf, isOutput=False)
    # h28: fp8 h2 for matmuls; aligned view + shifted views for [g1|h2] chunks.
    h28 = nc.declare_dram_parameter("h28", [Hp, nbt], f8, isOutput=False)
    wd = {}
    for name, k, m, dt_ in (
        ("Wzr1", 768, ZRM, f8), ("Wn1x", Fp, Hp, f8), ("Wn1h", Hp, Hp, f8),
        ("Wzr2", ZR2K, ZRM, f8), ("Wn2x", Hp, Hp, f8), ("Wn2h", Hp, Hp, f8),
        ("Wfc2T", Hp, FFp, bf), ("Wfc3T", FFp, FFp, bf), ("Wfc4T", FFp, Fp, bf),
    ):
        wd[name] = nc.declare_dram_parameter(name, [k, m], dt_, isOutput=False)
    biasT_d = nc.declare_dram_parameter("biasT", [128, BIAS_COLS], f32, isOutput=False)
    outT = nc.declare_dram_parameter("outT", [Fp, nbt], bf, isOutput=True)

    n_tiles = nbt // nb
    HC = Hp // 128  # 4 M-chunks per gate

    with tile.TileContext(nc) as tc:
        with (
            tc.tile_pool(name="wpool", bufs=1) as wpool,
            tc.tile_pool(name="bpool", bufs=1) as bpool,
            tc.tile_pool(name="io", bufs=2) as io,
            tc.tile_pool(name="inp", bufs=3) as inp,
            tc.tile_pool(name="act", bufs=3) as act,
            tc.tile_pool(name="psum", bufs=2, space="PSUM") as psum,
        ):
            # ACT-table warmup: first ScalarE transcendental carries the
            # ACT_TABLE_LOAD pseudo-inst; keep it off the critical chain.
            warm = bpool.tile([128, 1], f32, tag="warm")
            nc.vector.memset(warm, 0.0)
            nc.scalar.activation(warm, warm, AF.Sigmoid)

            W, BT = {}, {}

            def load_w(name, eng=None):
                dram = wd[name]
                k, m = dram.shape
                t = wpool.tile([128, k // 128, m], dram.dtype, tag=name)
                r = dram.rearrange("(c p) m -> p c m", p=128)
                for c in range(k // 128):
                    (eng or nc.sync).dma_start(out=t[:, c, :], in_=r[:, c, :])
                W[name] = t

            def load_bias():
                biasT = bpool.tile([128, BIAS_COLS], f32, tag="biasT")
                nc.sync.dma_start(out=biasT, in_=biasT_d[:, :])
                for _n, (_o, _c) in BIAS_LAYOUT.items():
                    BT[_n] = biasT[:, _o : _o + _c]

            xh_zr = xh8[0:768, :].rearrange("(c p) n -> p c n", p=128)
            h1m_al = xh8[257 : 257 + Hp, :].rearrange("(c p) n -> p c n", p=128)
            h1_bl = h1T.rearrange("(c p) n -> p c n", p=128)
            h2_bl = h2T.rearrange("(c p) n -> p c n", p=128)
            h28_al = h28.rearrange("(c p) n -> p c n", p=128)
            h2_s0 = h28[0:112, :]                     # -> partitions 16..127 of zr2 chunk 3
            h2_s1 = h28[112:496, :].rearrange("(c p) n -> p c n", p=128)
            outT_r = outT.rearrange("(c p) n -> p c n", p=128)

            def load_inputs(t):
                sl = ts(t, nb)
                xh = inp.tile([128, 6, nb], f8, tag="xh")      # zr1/nx1 K operand
                nc.sync.dma_start(out=xh, in_=xh_zr[:, :, sl])
                h1m = inp.tile([128, HC, nb], f8, tag="h1m")   # nh1 rhs (aligned h1)
                nc.sync.dma_start(out=h1m, in_=h1m_al[:, :, sl])
                h1s = inp.tile([128, HC, nb], bf, tag="h1s")   # blend h1
                nc.sync.dma_start(out=h1s, in_=h1_bl[:, :, sl])
                h2s = inp.tile([128, HC, nb], bf, tag="h2s")   # blend h2
                nc.sync.dma_start(out=h2s, in_=h2_bl[:, :, sl])
                h28s = inp.tile([128, HC, nb], f8, tag="h28s") # nh2 rhs (aligned h2)
                nc.sync.dma_start(out=h28s, in_=h28_al[:, :, sl])
                return xh, h1m, h1s, h2s, h28s

            tile0_inputs = load_inputs(0)

            # GRU1 weights share the sync ring with the input tiles; everything
            # needed later streams on the otherwise-idle PE ring in parallel.
            load_w("Wzr1")
            load_bias()
            for name in ("Wn1x", "Wn1h"):
                load_w(name)
            for name in ("Wzr2", "Wn2x", "Wn2h", "Wfc2T", "Wfc3T", "Wfc4T"):
                load_w(name, eng=nc.scalar)

            def matseq(ps, pairs):
                n = len(pairs)
                for i, (lhsT, rhs) in enumerate(pairs):
                    nc.tensor.matmul(ps, lhsT, rhs, start=(i == 0), stop=(i == n - 1))

            DR = mybir.MatmulPerfMode.DoubleRow

            def matseq_dr(ps, Wt, kc, col, mw, rhs_t):
                """fp8 accumulation over kc K-chunks of [128, kc, *] tiles using
                DoubleRow on consecutive chunk pairs (odd tail chunk = normal)."""
                n = (kc + 1) // 2
                for i in range(n):
                    k = 2 * i
                    if k + 2 <= kc:
                        nc.tensor.matmul(
                            ps, Wt[:, k : k + 2, col : col + mw],
                            rhs_t[:, k : k + 2, :],
                            start=(i == 0), stop=(i == n - 1), perf_mode=DR)
                    else:
                        nc.tensor.matmul(
                            ps, Wt[:, k, col : col + mw], rhs_t[:, k, :],
                            start=(i == 0), stop=(i == n - 1))

            def gru(zr_t, kzr, Wzr, nx_t, knx, Wnx, nh_t, Wnh, h_al,
                    bzr, bnx, bnh, out_chunk):
                """One GRU step, all matmuls fp8/DoubleRow. zr_t/nx_t/nh_t are
                [128, kc, nb] fp8 rhs tiles; h_al: bf16 blend chunks.
                out_chunk(m) -> output AP for chunk m."""
                # z,r preactivations: 7 contiguous M chunks (chunk 6 is 32 wide)
                zro = act.tile([128, ZRC, nb], bf, tag="zro")
                for c in range(ZRC):
                    mw = min(128, ZRM - c * 128)
                    ps = psum.tile([128, nb], f32, tag="ps_zr")
                    matseq_dr(ps[:mw, :], Wzr, kzr, c * 128, mw, zr_t)
                    nc.scalar.activation(zro[:mw, c, :], ps[:mw, :], AF.Sigmoid,
                                         bias=bzr[:mw, c : c + 1])
                # realign r (features at concat rows 400+f) to h's lanes
                r_al = act.tile([128, HC, nb], bf, tag="r_al")
                for m in range(3):
                    nc.scalar.dma_start(out=r_al[0:112, m, :], in_=zro[16:128, 3 + m, :])
                    nc.scalar.dma_start(out=r_al[112:128, m, :], in_=zro[0:16, 4 + m, :])
                nc.scalar.dma_start(out=r_al[0:16, 3, :], in_=zro[16:32, 6, :])

                for m in range(HC):
                    pz = 128 if m < 3 else 16   # valid rows of this chunk
                    col = m * 128
                    psx = psum.tile([128, nb], f32, tag="ps_nx")
                    matseq_dr(psx, Wnx, knx, col, 128, nx_t)
                    psh = psum.tile([128, nb], f32, tag="ps_nh")
                    matseq_dr(psh, Wnh, HC, col, 128, nh_t)
                    # rhn = (psh + bnh) * r ; npre = (psx + bnx) + rhn ; n = tanh(npre)
                    rhn = act.tile([128, nb], f32, tag="rhn")
                    nc.vector.scalar_tensor_tensor(
                        rhn[:pz, :], psh[:pz, :], bnh[:pz, m : m + 1],
                        r_al[:pz, m, :], op0=ALU.add, op1=ALU.mult)
                    npre = act.tile([128, nb], f32, tag="npre")
                    nc.vector.scalar_tensor_tensor(
                        npre[:pz, :], psx[:pz, :], bnx[:pz, m : m + 1],
                        rhn[:pz, :], op0=ALU.add, op1=ALU.add)
                    n_t = act.tile([128, nb], bf, tag="n_t")
                    nc.scalar.activation(n_t[:pz, :], npre[:pz, :], AF.Tanh)
                    # h' = n + z*(h - n);  z chunk m lives in zro (contig layout)
                    z_ap = zro[:pz, m, :] if m < 3 else zro[0:16, 3, :]
                    d = act.tile([128, nb], bf, tag="d")
                    nc.vector.tensor_sub(d[:pz, :], h_al[m][:pz, :], n_t[:pz, :])
                    zd = act.tile([128, nb], bf, tag="zd")
                    nc.vector.tensor_mul(zd[:pz, :], z_ap, d[:pz, :])
                    out_ap = out_chunk(m)
                    p = min(out_ap.shape[0], pz)
                    nc.vector.tensor_add(out_ap[:p, :] if out_ap.shape[0] > p else out_ap,
                                         n_t[:p, :], zd[:p, :])

            def fc(in_ks, Wt, bias, mc, kind, out_tag):
                outs = io.tile([128, mc, nb], bf, tag=out_tag)
                for m in range(mc):
                    ps = psum.tile([128, nb], f32, tag="ps_fc")
                    matseq(ps, [(Wt[:, k, m * 128 : (m + 1) * 128], rhs)
                                for k, rhs in enumerate(in_ks)])
                    if kind == "relu":
                        nc.vector.tensor_scalar(
                            outs[:, m, :], ps, bias[:, m : m + 1], 0.0,
                            op0=ALU.add, op1=ALU.max)
                    else:
                        nc.scalar.activation(outs[:, m, :], ps, AF.Sigmoid,
                                             bias=bias[:, m : m + 1])
                return outs

            for t in range(n_tiles):
                sl = ts(t, nb)
                xh, h1m, h1s, h2s, h28s = \
                    tile0_inputs if t == 0 else load_inputs(t)

                # zr2op = GRU2's [g1|h2] fp8 operand (one tile so DoubleRow can
                # pair consecutive chunks): chunks 0..2 + [0:16] of chunk 3 are
                # written by GRU1's blend (fp8 out); the rest comes from h28.
                zr2op = io.tile([128, ZRC, nb], f8, tag="zr2op")
                nc.sync.dma_start(out=zr2op[16:128, 3, :], in_=h2_s0[:, sl])
                nc.sync.dma_start(out=zr2op[:, 4:7, :], in_=h2_s1[:, :, sl])

                def g1_out(m):
                    return zr2op[:, m, :] if m < 3 else zr2op[0:16, 3, :]

                h1_ks = [h1s[:, c, :] for c in range(HC)]
                gru(xh, 6, W["Wzr1"], xh, 3, W["Wn1x"], h1m, W["Wn1h"], h1_ks,
                    BT["bzr1"], BT["bnx1"], BT["bnh1"], g1_out)

                h2_ks = [h2s[:, c, :] for c in range(HC)]
                g2 = io.tile([128, HC, nb], bf, tag="g2")
                # g2 pad rows (feature >= 400 of chunk 3) must be finite for
                # fc2's zero-weight contraction: zero them once per tile.
                nc.gpsimd.memset(g2[:, 3, :], 0.0)
                gru(zr2op, ZRC, W["Wzr2"], zr2op, HC, W["Wn2x"], h28s, W["Wn2h"], h2_ks,
                    BT["bzr2"], BT["bnx2"], BT["bnh2"],
                    lambda m: g2[:, m, :])

                g2_ks = [g2[:, c, :] for c in range(HC)]
                f2 = fc(g2_ks, W["Wfc2T"], BT["bfc2"], FFp // 128, "relu", "f2")
                f3 = fc([f2[:, c, :] for c in range(FFp // 128)],
                        W["Wfc3T"], BT["bfc3"], FFp // 128, "relu", "f3")
                o = fc([f3[:, c, :] for c in range(FFp // 128)],
                       W["Wfc4T"], BT["bfc4"], Fp // 128, "sig", "o")
                nc.sync.dma_start(out=outT_r[:, :, sl], in_=o)

    nc.compile()
    return nc


def _shard_inputs(inp, weights, biases):
    x = np.asarray(inp["x"], dtype=np.float32)
    h1 = np.asarray(inp["h1"], dtype=np.float32)
    h2 = np.asarray(inp["h2"], dtype=np.float32)

    xh8 = np.zeros((NCORES, XH1, BPC), dtype=FP8)    # matmul operand [x|h1]
    h1T = np.zeros((NCORES, Hp, BPC), dtype=BF16)    # blend h1
    h2T = np.zeros((NCORES, Hp, BPC), dtype=BF16)    # blend h2
    h28 = np.zeros((NCORES, Hp, BPC), dtype=FP8)     # matmul h2
    for i in range(NCORES):
        sl = slice(i * BPC, (i + 1) * BPC)
        xh8[i, :F] = x[sl].T.astype(FP8)
        xh8[i, F : F + H] = h1[sl].T.astype(FP8)
        h1T[i, :H] = h1[sl].T.astype(BF16)
        h2T[i, :H] = h2[sl].T.astype(BF16)
        h28[i, :H] = h2[sl].T.astype(FP8)

    in_maps = []
    for i in range(NCORES):
        m = {"xh8": xh8[i], "h1T": h1T[i], "h2T": h2T[i], "h28": h28[i]}
        m.update(weights)
        m.update(biases)
        in_maps.append(m)
    return in_maps


def _run(inp, trace=False):
    weights, biases = prepare_weights(inp)
    nc = build_nc()
    in_maps = _shard_inputs(inp, weights, biases)
    res = run_bass_kernel_spmd(nc, in_maps, list(range(NCORES)), trace=trace)
    out = np.empty((B, F), dtype=np.float32)
    for i in range(NCORES):
        out[i * BPC : (i + 1) * BPC] = (
            np.asarray(res.results[i]["outT"][:F]).astype(np.float32).T
        )
    return out, res


def kernel(**inputs) -> np.ndarray:
    out, _ = _run(inputs, trace=False)
    return out



# revision 5
# speedup vs baseline: 1.1010x; 1.1010x over previous
"""NsNet2 single-step (fc1 + 2x GRU cell + 3x FC) Trainium2 kernel.

Strategy:
  - Pure data parallel: batch B=32768 sharded as 4096 rows per NeuronCore (8 cores).
  - Feature-major ("transposed") layout on chip: activations live as [feat, batch]
    so every matmul's moving operand is already in [K, N] form -> zero on-chip
    transposes. Host transposes inputs/outputs (free; not on HW critical path).
  - ALL matmuls fp8 DoubleRow (GRU gates and the 3 FC layers) with fp32 PSUM
    accumulation; fp32 biases fused into ScalarE activation (sigmoid/tanh) or
    VectorE tensor_scalar (relu).
  - fc1 is folded into the GRU1 input-gate weights on the host (fc1 is linear and
    f1 is consumed only by GRU1's input matmuls):  (x@Wfc1.T+b) @ Wg.T =
    x @ (Wg@Wfc1).T + (Wg@b + bg).
  - z,r gates sum their input-side and hidden-side matmuls in one PSUM, so their
    contraction operands are K-concatenated ([x|h1] resp. [g1|h2]).
  - 3-stage software pipeline across batch tiles: per group emit
    A(t+2)=GRU1, B(t+1)=GRU2, C(t)=FC chain, so the in-order PE queue never
    stalls on a same-tile blend chain (every consumer's producer finished
    >=1 full group of PE work earlier).
  - PSUM tags zr/nx/nh shared between GRU1 and GRU2 stages + fc tag: 4 tags x
    2 bufs = exactly 8 PSUM banks.
  - Elementwise blend (h' = n + z*(h-n)) split: sub/mul on GpSimd (SBUF-only
    ops), everything touching PSUM on DVE, transcendentals on ScalarE.
"""

import os
import sys

import numpy as np
import ml_dtypes

sys.path.insert(0, "/opt/trn_rl_repo")

import concourse.bacc as bacc
import concourse.bass as bass
import concourse.mybir as mybir
import concourse.tile as tile
from concourse.bass import ts
from concourse.bass_utils import run_bass_kernel_spmd

BF16 = ml_dtypes.bfloat16
FP8 = ml_dtypes.float8_e4m3

B, F, H, FF = 32768, 257, 400, 600
NCORES = 8
BPC = B // NCORES          # 4096 batch rows per core
Hp, FFp, Fp = 512, 640, 384  # padded feature dims
XH1 = 769                  # [x(257) | h1(400) | pad(112)] rows; 6 zr chunks + aligned h1 view at 257
ZR2K = 896                 # [g1(400) | h2(400) | pad(96)] -> 7 chunks
ZRM = 800                  # contiguous [z(400) | r(400)] output cols -> 7 M chunks
ZRC = 7
NB = 512                   # matmul free-dim tile (one PSUM bank of fp32)

AF = mybir.ActivationFunctionType
ALU = mybir.AluOpType

# packed bias column layout: name -> (offset, n_chunks)
BIAS_LAYOUT = {}
_off = 0
for _n, _c in (("bzr1", 7), ("bnx1", 4), ("bnh1", 4),
               ("bzr2", 7), ("bnx2", 4), ("bnh2", 4),
               ("bfc2", 5), ("bfc3", 5), ("bfc4", 3)):
    BIAS_LAYOUT[_n] = (_off, _c)
    _off += _c
BIAS_COLS = _off


def _pad2(a, rows, cols):
    out = np.zeros((rows, cols), dtype=np.float64)
    out[: a.shape[0], : a.shape[1]] = a
    return out


def _bias_tile(vec, padded):
    """Pack a [padded] bias vector as [128, padded//128] fp32 (partition-major)."""
    v = np.zeros(padded, dtype=np.float64)
    v[: vec.shape[0]] = vec
    return np.ascontiguousarray(v.reshape(padded // 128, 128).T).astype(np.float32)


def prepare_weights(inp):
    f64 = {k: np.asarray(v, dtype=np.float64) for k, v in inp.items()}
    w = {}

    # fc1 fold for GRU1 input side
    Wx = {}
    bx = {}
    for name in ("z", "r", "n"):
        Wx[name] = (f64[f"Wi{name}1"] @ f64["Wfc1"]).T          # [F, H]
        bx[name] = f64[f"bi{name}1"] + f64[f"Wi{name}1"] @ f64["bfc1"]

    # GRU1 z,r: K-concat [x(257) | h1(400)] -> rows 0..656 of XH1 space,
    # M = contiguous [z(400) | r(400)] = 800 -> 7 chunks; r is lane-realigned
    # on chip by a small SBUF->SBUF DMA after the sigmoid.
    Wzr1 = np.zeros((768, ZRM), dtype=np.float64)
    for g, name in enumerate(("z", "r")):
        Wzr1[:F, g * H : g * H + H] = Wx[name]
        Wzr1[F : F + H, g * H : g * H + H] = f64[f"Wh{name}1"].T
    w["Wzr1"] = Wzr1
    # GRU1 n input side: K = x chunks of XH1 (rows 0..383; rows 257+ are h1 -> zero)
    w["Wn1x"] = _pad2(Wx["n"], Fp, Hp)
    # GRU1 n hidden side: aligned h1 (XH1 rows 257..768)
    w["Wn1h"] = _pad2(f64["Whn1"].T, Hp, Hp)

    # GRU2 z,r: K-concat [g1(400) | h2(400)] -> 800 rows -> 7 chunks
    Wzr2 = np.zeros((ZR2K, ZRM), dtype=np.float64)
    for g, name in enumerate(("z", "r")):
        Wzr2[:H, g * H : g * H + H] = f64[f"Wi{name}2"].T
        Wzr2[H : 2 * H, g * H : g * H + H] = f64[f"Wh{name}2"].T
    w["Wzr2"] = Wzr2
    # GRU2 n input side: K = g1 aligned (4 chunks; chunk 3 partitions 16.. are h2 -> zero)
    w["Wn2x"] = _pad2(f64["Win2"].T, Hp, Hp)
    w["Wn2h"] = _pad2(f64["Whn2"].T, Hp, Hp)

    w["Wfc2T"] = _pad2(f64["Wfc2"].T, Hp, FFp)    # [512, 640]
    w["Wfc3T"] = _pad2(f64["Wfc3"].T, FFp, FFp)   # [640, 640]
    w["Wfc4T"] = _pad2(f64["Wfc4"].T, FFp, Fp)    # [640, 384]

    weights = {
        k: np.ascontiguousarray(v).astype(FP8)
        for k, v in w.items()
    }

    parts = [
        ("bzr1", _bias_tile(np.concatenate([bx["z"] + f64["bhz1"],
                                            bx["r"] + f64["bhr1"]]), 896)),
        ("bnx1", _bias_tile(bx["n"], Hp)),
        ("bnh1", _bias_tile(f64["bhn1"], Hp)),
        ("bzr2", _bias_tile(np.concatenate([f64["biz2"] + f64["bhz2"],
                                            f64["bir2"] + f64["bhr2"]]), 896)),
        ("bnx2", _bias_tile(f64["bin2"], Hp)),
        ("bnh2", _bias_tile(f64["bhn2"], Hp)),
        ("bfc2", _bias_tile(f64["bfc2"], FFp)),
        ("bfc3", _bias_tile(f64["bfc3"], FFp)),
        ("bfc4", _bias_tile(f64["bfc4"], Fp)),
    ]
    biases = {"biasT": np.concatenate([p[1] for p in parts], axis=1)}
    return weights, biases


def build_nc(nbt=BPC, nb=NB):
    """Build the per-core Bass program. nbt = per-core batch, nb = free-dim tile."""
    nc = bacc.Bacc("TRN2", target_bir_lowering=False, debug=False)
    bf = mybir.dt.bfloat16
    f32 = mybir.dt.float32

    f8 = mybir.dt.float8e4

    # xh8 rows: 0..256 = x.T, 257..656 = h1.T, 657..768 = zeros (fp8 matmul
    # operand). zr view = rows 0..767 (6 chunks); aligned-h1 view = 257..768.
    xh8 = nc.declare_dram_parameter("xh8", [XH1, nbt], f8, isOutput=False)
    h1T = nc.declare_dram_parameter("h1T", [Hp, nbt], bf, isOutput=False)
    h2T = nc.declare_dram_parameter("h2T", [Hp, nbt], bf, isOutput=False)
    # h28: fp8 h2 for matmuls; aligned view + shifted views for [g1|h2] chunks.
    h28 = nc.declare_dram_parameter("h28", [Hp, nbt], f8, isOutput=False)
    wd = {}
    for name, k, m in (
        ("Wzr1", 768, ZRM), ("Wn1x", Fp, Hp), ("Wn1h", Hp, Hp),
        ("Wzr2", ZR2K, ZRM), ("Wn2x", Hp, Hp), ("Wn2h", Hp, Hp),
        ("Wfc2T", Hp, FFp), ("Wfc3T", FFp, FFp), ("Wfc4T", FFp, Fp),
    ):
        wd[name] = nc.declare_dram_parameter(name, [k, m], f8, isOutput=False)
    biasT_d = nc.declare_dram_parameter("biasT", [128, BIAS_COLS], f32, isOutput=False)
    outT = nc.declare_dram_parameter("outT", [Fp, nbt], bf, isOutput=True)

    n_tiles = nbt // nb
    HC = Hp // 128  # 4 M-chunks per gate

    with tile.TileContext(nc) as tc:
        with (
            tc.tile_pool(name="wpool", bufs=1) as wpool,
            tc.tile_pool(name="bpool", bufs=1) as bpool,
            tc.tile_pool(name="io", bufs=3) as io,
            tc.tile_pool(name="cio", bufs=2) as cio,
            tc.tile_pool(name="inp", bufs=4) as inp,
            tc.tile_pool(name="act", bufs=3) as act,
            tc.tile_pool(name="psum", bufs=2, space="PSUM") as psum,
        ):
            # ACT-table warmup: first ScalarE transcendental carries the
            # ACT_TABLE_LOAD pseudo-inst; keep it off the critical chain.
            warm = bpool.tile([128, 1], f32, tag="warm")
            nc.vector.memset(warm, 0.0)
            nc.scalar.activation(warm, warm, AF.Sigmoid)
            warm2 = bpool.tile([128, 1], f32, tag="warm2")
            nc.vector.memset(warm2, 0.0)
            nc.scalar.activation(warm2, warm2, AF.Tanh)

            W, BT = {}, {}

            def load_w(name, eng, lo=0, hi=None):
                dram = wd[name]
                k, m = dram.shape
                if name in W:
                    t = W[name]
                else:
                    t = wpool.tile([128, k // 128, m], dram.dtype, tag=name)
                    W[name] = t
                r = dram.rearrange("(c p) m -> p c m", p=128)
                for c in range(lo, k // 128 if hi is None else hi):
                    eng.dma_start(out=t[:, c, :], in_=r[:, c, :])

            def load_bias():
                biasT = bpool.tile([128, BIAS_COLS], f32, tag="biasT")
                nc.scalar.dma_start(out=biasT, in_=biasT_d[:, :])
                for _n, (_o, _c) in BIAS_LAYOUT.items():
                    BT[_n] = biasT[:, _o : _o + _c]

            xh_zr = xh8[0:768, :].rearrange("(c p) n -> p c n", p=128)
            h1m_al = xh8[257 : 257 + Hp, :].rearrange("(c p) n -> p c n", p=128)
            h1_bl = h1T.rearrange("(c p) n -> p c n", p=128)
            h2_bl = h2T.rearrange("(c p) n -> p c n", p=128)
            h28_al = h28.rearrange("(c p) n -> p c n", p=128)
            h2_s0 = h28[0:112, :]                     # -> partitions 16..127 of zr2 chunk 3
            h2_s1 = h28[112:496, :].rearrange("(c p) n -> p c n", p=128)
            outT_r = outT.rearrange("(c p) n -> p c n", p=128)

            def load_inputs(t, first=False):
                sl = ts(t, nb)
                xh = inp.tile([128, 6, nb], f8, tag="xh")      # zr1/nx1 K operand
                nc.sync.dma_start(out=xh, in_=xh_zr[:, :, sl])
                h1m = inp.tile([128, HC, nb], f8, tag="h1m")   # nh1 rhs (aligned h1)
                nc.sync.dma_start(out=h1m, in_=h1m_al[:, :, sl])
                if first:
                    # GRU1 weights land before the bulkier blend operands so
                    # tile-0 matmuls can start asap (DMA-capable queues are
                    # sync/scalar/gpsimd only).
                    load_w("Wzr1", nc.sync, 0, 3)
                    load_w("Wzr1", nc.gpsimd, 3, 6)
                    load_w("Wn1x", nc.scalar)
                    load_w("Wn1h", nc.scalar)
                h1s = inp.tile([128, HC, nb], bf, tag="h1s")   # blend h1
                nc.sync.dma_start(out=h1s, in_=h1_bl[:, :, sl])
                h2s = inp.tile([128, HC, nb], bf, tag="h2s")   # blend h2
                nc.sync.dma_start(out=h2s, in_=h2_bl[:, :, sl])
                h28s = inp.tile([128, HC, nb], f8, tag="h28s") # nh2 rhs (aligned h2)
                nc.sync.dma_start(out=h28s, in_=h28_al[:, :, sl])
                return xh, h1m, h1s, h2s, h28s

            DR = mybir.MatmulPerfMode.DoubleRow

            def matseq_dr(ps, Wt, kc, col, mw, rhs_t):
                """fp8 accumulation over kc K-chunks of [128, kc, *] tiles using
                DoubleRow on consecutive chunk pairs (odd tail chunk = normal)."""
                n = (kc + 1) // 2
                for i in range(n):
                    k = 2 * i
                    if k + 2 <= kc:
                        nc.tensor.matmul(
                            ps, Wt[:, k : k + 2, col : col + mw],
                            rhs_t[:, k : k + 2, :],
                            start=(i == 0), stop=(i == n - 1), perf_mode=DR)
                    else:
                        nc.tensor.matmul(
                            ps, Wt[:, k, col : col + mw], rhs_t[:, k, :],
                            start=(i == 0), stop=(i == n - 1))

            def gru(zr_t, kzr, Wzr, nx_t, knx, Wnx, nh_t, Wnh, h_al,
                    bzr, bnx, bnh, out_chunk):
                """One GRU step, all matmuls fp8/DoubleRow. zr_t/nx_t/nh_t are
                [128, kc, nb] fp8 rhs tiles; h_al: bf16 blend chunks.
                out_chunk(m) -> output AP for chunk m."""
                # z,r preactivations: 7 contiguous M chunks (chunk 6 is 32 wide)
                zro = act.tile([128, ZRC, nb], bf, tag="zro")
                for c in range(ZRC):
                    mw = min(128, ZRM - c * 128)
                    ps = psum.tile([128, nb], f32, tag="ps_zr")
                    matseq_dr(ps[:mw, :], Wzr, kzr, c * 128, mw, zr_t)
                    nc.scalar.activation(zro[:mw, c, :], ps[:mw, :], AF.Sigmoid,
                                         bias=bzr[:mw, c : c + 1])
                # realign r (features at concat rows 400+f) to h's lanes
                r_al = act.tile([128, HC, nb], bf, tag="r_al")
                for m in range(3):
                    nc.gpsimd.dma_start(out=r_al[0:112, m, :], in_=zro[16:128, 3 + m, :])
                    nc.gpsimd.dma_start(out=r_al[112:128, m, :], in_=zro[0:16, 4 + m, :])
                nc.gpsimd.dma_start(out=r_al[0:16, 3, :], in_=zro[16:32, 6, :])

                for m in range(HC):
                    pz = 128 if m < 3 else 16   # valid rows of this chunk
                    col = m * 128
                    psx = psum.tile([128, nb], f32, tag="ps_nx")
                    matseq_dr(psx, Wnx, knx, col, 128, nx_t)
                    psh = psum.tile([128, nb], f32, tag="ps_nh")
                    matseq_dr(psh, Wnh, HC, col, 128, nh_t)
                    # rhn = (psh + bnh) * r ; npre = (psx + bnx) + rhn ; n = tanh(npre)
                    rhn = act.tile([128, nb], f32, tag="rhn")
                    nc.vector.scalar_tensor_tensor(
                        rhn[:pz, :], psh[:pz, :], bnh[:pz, m : m + 1],
                        r_al[:pz, m, :], op0=ALU.add, op1=ALU.mult)
                    npre = act.tile([128, nb], f32, tag="npre")
                    nc.vector.scalar_tensor_tensor(
                        npre[:pz, :], psx[:pz, :], bnx[:pz, m : m + 1],
                        rhn[:pz, :], op0=ALU.add, op1=ALU.add)
                    n_t = act.tile([128, nb], bf, tag="n_t")
                    nc.scalar.activation(n_t[:pz, :], npre[:pz, :], AF.Tanh)
                    # h' = n + z*(h - n);  z chunk m lives in zro (contig layout)
                    z_ap = zro[:pz, m, :] if m < 3 else zro[0:16, 3, :]
                    # sub/mul are SBUF-only -> GpSimd (keeps DVE for PSUM ops);
                    # the 16-partition tail chunk stays on DVE (GpSimd would
                    # engage only one Q7 core).
                    ew = nc.gpsimd if m < 3 else nc.vector
                    d = act.tile([128, nb], bf, tag="d")
                    ew.tensor_tensor(d[:pz, :], h_al[m][:pz, :], n_t[:pz, :],
                                     op=ALU.subtract)
                    zd = act.tile([128, nb], bf, tag="zd")
                    ew.tensor_tensor(zd[:pz, :], z_ap, d[:pz, :], op=ALU.mult)
                    out_ap = out_chunk(m)
                    p = min(out_ap.shape[0], pz)
                    nc.vector.tensor_add(out_ap[:p, :] if out_ap.shape[0] > p else out_ap,
                                         n_t[:p, :], zd[:p, :])

            def fc(in_t, kc, Wt, bias, mc, kind, out_tag, out_dt):
                outs = cio.tile([128, mc, nb], out_dt, tag=out_tag)
                for m in range(mc):
                    ps = psum.tile([128, nb], f32, tag="ps_fc")
                    matseq_dr(ps, Wt, kc, m * 128, 128, in_t)
                    if kind == "relu":
                        nc.vector.tensor_scalar(
                            outs[:, m, :], ps, bias[:, m : m + 1], 0.0,
                            op0=ALU.add, op1=ALU.max)
                    else:
                        nc.scalar.activation(outs[:, m, :], ps, AF.Sigmoid,
                                             bias=bias[:, m : m + 1])
                return outs

            # ---- pipeline stages ----
            st = {}  # t -> dict of live tiles

            def stage_A(t):
                """GRU1 for tile t: consumes inputs, produces zr2op (the
                [g1|h2] fp8 operand for GRU2)."""
                sl = ts(t, nb)
                xh, h1m, h1s, h2s, h28s = st[t]["in"]
                zr2op = io.tile([128, ZRC, nb], f8, tag="zr2op")
                nc.sync.dma_start(out=zr2op[16:128, 3, :], in_=h2_s0[:, sl])
                nc.sync.dma_start(out=zr2op[:, 4:7, :], in_=h2_s1[:, :, sl])

                def g1_out(m):
                    return zr2op[:, m, :] if m < 3 else zr2op[0:16, 3, :]

                h1_ks = [h1s[:, c, :] for c in range(HC)]
                gru(xh, 6, W["Wzr1"], xh, 3, W["Wn1x"], h1m, W["Wn1h"], h1_ks,
                    BT["bzr1"], BT["bnx1"], BT["bnh1"], g1_out)
                st[t]["zr2op"] = zr2op

            def stage_B(t):
                """GRU2 for tile t: consumes zr2op + h2 operands, produces g2
                (fp8, fc2's K operand)."""
                zr2op = st[t]["zr2op"]
                _, _, _, h2s, h28s = st[t]["in"]
                h2_ks = [h2s[:, c, :] for c in range(HC)]
                g2 = io.tile([128, HC, nb], f8, tag="g2")
                # g2 pad rows (feature >= 400 of chunk 3) must be finite for
                # fc2's zero-weight contraction: zero them once per tile.
                nc.gpsimd.memset(g2[:, 3, :], 0.0)
                gru(zr2op, ZRC, W["Wzr2"], zr2op, HC, W["Wn2x"], h28s, W["Wn2h"],
                    h2_ks, BT["bzr2"], BT["bnx2"], BT["bnh2"],
                    lambda m: g2[:, m, :])
                st[t]["g2"] = g2

            def stage_C(t):
                """FC chain for tile t: g2 -> relu fc2 -> relu fc3 -> sigmoid
                fc4 -> DMA out."""
                sl = ts(t, nb)
                g2 = st[t]["g2"]
                f2 = fc(g2, HC, W["Wfc2T"], BT["bfc2"], FFp // 128, "relu",
                        "f2", f8)
                f3 = fc(f2, FFp // 128, W["Wfc3T"], BT["bfc3"], FFp // 128,
                        "relu", "f3", f8)
                o = fc(f3, FFp // 128, W["Wfc4T"], BT["bfc4"], Fp // 128,
                       "sig", "o", bf)
                nc.sync.dma_start(out=outT_r[:, :, sl], in_=o)
                del st[t]

            # ---- emission: software-pipelined interleave ----
            # prologue
            st[0] = {"in": load_inputs(0, first=True)}
            load_bias()
            # GRU2 weights stream on the gpsimd queue, which is idle early
            load_w("Wzr2", nc.gpsimd)
            load_w("Wn2x", nc.gpsimd)
            load_w("Wn2h", nc.gpsimd)
            st[1] = {"in": load_inputs(1)}
            stage_A(0)
            # fc weights interleave on the scalar queue between early stages:
            # behind tile-0/1 activations (so they don't delay them) but well
            # before C(0) consumes them.
            load_w("Wfc2T", nc.scalar)
            st[2] = {"in": load_inputs(2)}
            stage_A(1)
            load_w("Wfc3T", nc.scalar)
            stage_B(0)
            load_w("Wfc4T", nc.scalar)
            # steady state: per group emit A(t+2), B(t+1), C(t)
            for t in range(n_tiles - 2):
                if t + 3 < n_tiles:
                    st[t + 3] = {"in": load_inputs(t + 3)}
                stage_A(t + 2)
                stage_B(t + 1)
                stage_C(t)
            # epilogue
            stage_B(n_tiles - 1)
            stage_C(n_tiles - 2)
            stage_C(n_tiles - 1)

    nc.compile()
    return nc


def _shard_inputs(inp, weights, biases):
    x = np.asarray(inp["x"], dtype=np.float32)
    h1 = np.asarray(inp["h1"], dtype=np.float32)
    h2 = np.asarray(inp["h2"], dtype=np.float32)

    xh8 = np.zeros((NCORES, XH1, BPC), dtype=FP8)    # matmul operand [x|h1]
    h1T = np.zeros((NCORES, Hp, BPC), dtype=BF16)    # blend h1
    h2T = np.zeros((NCORES, Hp, BPC), dtype=BF16)    # blend h2
    h28 = np.zeros((NCORES, Hp, BPC), dtype=FP8)     # matmul h2
    for i in range(NCORES):
        sl = slice(i * BPC, (i + 1) * BPC)
        xh8[i, :F] = x[sl].T.astype(FP8)
        xh8[i, F : F + H] = h1[sl].T.astype(FP8)
        h1T[i, :H] = h1[sl].T.astype(BF16)
        h2T[i, :H] = h2[sl].T.astype(BF16)
        h28[i, :H] = h2[sl].T.astype(FP8)

    in_maps = []
    for i in range(NCORES):
        m = {"xh8": xh8[i], "h1T": h1T[i], "h2T": h2T[i], "h28": h28[i]}
        m.update(weights)
        m.update(biases)
        in_maps.append(m)
    return in_maps


def _run(inp, trace=False):
    weights, biases = prepare_weights(inp)
    nc = build_nc()
    in_maps = _shard_inputs(inp, weights, biases)
    res = run_bass_kernel_spmd(nc, in_maps, list(range(NCORES)), trace=trace)
    out = np.empty((B, F), dtype=np.float32)
    for i in range(NCORES):
        out[i * BPC : (i + 1) * BPC] = (
            np.asarray(res.results[i]["outT"][:F]).astype(np.float32).T
        )
    return out, res


def kernel(**inputs) -> np.ndarray:
    out, _ = _run(inputs, trace=False)
    return out
